# revision 4
# baseline (speedup 1.0000x reference)
"""Trainium2 Bass kernel: low-rank (LoRA-style) linear with 2:4 soft-threshold
pruned weights, fp16 matmul / fp32 accumulate.

  wA = soft_threshold24(weight_A) * scale_A          # [IN, R]
  wB = soft_threshold24(weight_B) * scale_B          # [OUT, R]
  x_proj = f16(x) @ f16(wA)            (f32 accum)   # [N, R]
  out    = f16(x_proj) @ f16(wB).T + bias            # [N, OUT]

Sharding: data-parallel over the token dim across 8 cores (2048 tokens/core),
small weights replicated. No collectives.

I/O strategy: the reference itself computes both matmuls on f16 operands, so
the host casts x / weights / bias to f16 before upload and upcasts the f16
result to f32 after download.  This halves HBM traffic (the kernel is
memory-bound) and matches reference numerics.

Per-core pipeline (4 groups of 512 tokens):
  x is loaded ALREADY TRANSPOSED via the DMA XBAR transpose (HWDGE,
  DRAM->SBUF, f16): per group g and input-chunk c, xT[c][:, t] = x[t, c*128+:].
  This removes all PE transposes and their PSUM->SBUF copies.
  mm1: 32 accumulating f16 matmuls vs wa16 -> ps1[64, 512]
  xpa: ACT cast copy PSUM->SBUF f16 + ones row (bias trick)
  mm2: per 128-token tile, 8 f16 matmuls vs wbt (bias row folded in)
  out: PSUM->SBUF f16 copies alternating DVE/ACT, row-contiguous f16 store.
"""

import sys

import numpy as np

if "/opt/trn_rl_repo" not in sys.path:
    sys.path.insert(0, "/opt/trn_rl_repo")

B, S, IN_F, OUT_F, RANK = 4, 4096, 4096, 4096, 64
N_CORES = 8
N_TOK = B * S                   # 16384
T_CORE = N_TOK // N_CORES       # 2048 tokens per core
P = 128
GTOK = 512                      # tokens per group
N_GRP = T_CORE // GTOK          # 4 groups per core
TPG = GTOK // P                 # 4 token tiles per group
N_IB = IN_F // P                # 32 input-feature chunks
MM2_N = 512
N_OB = OUT_F // MM2_N           # 8 output column groups

_CACHE = {}


def _soft_threshold_weights(nc, pool, w_dram, scale, out_f16):
    """Emit IR computing out_f16 = soft_threshold24(w_dram f16) * scale.

    w_dram: [IN_or_OUT, RANK] f16, viewed as [P, blocks, RANK] with
    partition = row-within-block.  Returns the thresholded f16 tile
    [P, blocks, RANK] (also written to out_f16 when given).
    """
    import concourse.mybir as mybir

    f16 = mybir.dt.float16
    nb = w_dram.shape[0] // P
    wf = pool.tile([P, nb, RANK], f16, tag="wstage", name="wstage")
    nc.scalar.dma_start(wf[:], w_dram[:].rearrange("(b p) r -> p b r", p=P))

    thr = pool.tile([P, nb, RANK], f16, tag="wthr", name="wthr")
    amin = mybir.AluOpType.min
    amx = mybir.AluOpType.max
    ve = nc.vector

    wfh = wf[:]
    g = wfh.rearrange("p b (g q) -> p b g q", q=4)
    gj = [g[:, :, :, j : j + 1] for j in range(4)]
    ash = [P, nb, RANK // 4, 1]
    wneg = pool.tile([P, nb, RANK], f16, tag="wneg", name="wneg")
    ve.tensor_scalar_mul(wneg[:], wfh, -1.0)
    ng = wneg[:].rearrange("p b (g q) -> p b g q", q=4)
    ab = [pool.tile(ash, f16, tag=f"abs{j}", name=f"abs{j}") for j in range(4)]
    for j in range(4):
        ve.tensor_tensor(ab[j][:], gj[j], ng[:, :, :, j : j + 1], op=amx)
    m1 = pool.tile(ash, f16, tag="m1", name="m1")
    M1 = pool.tile(ash, f16, tag="M1", name="M1")
    m2 = pool.tile(ash, f16, tag="abs0", name="m2")
    M2 = pool.tile(ash, f16, tag="abs1", name="M2")
    ve.tensor_tensor(m1[:], ab[0][:], ab[1][:], op=amin)
    ve.tensor_tensor(M1[:], ab[0][:], ab[1][:], op=amx)
    ve.tensor_tensor(m2[:], ab[2][:], ab[3][:], op=amin)
    ve.tensor_tensor(M2[:], ab[2][:], ab[3][:], op=amx)
    # 2nd smallest of the 4 = min(max(m1, m2), min(M1, M2))
    t = pool.tile(ash, f16, tag="abs2", name="t")
    ve.tensor_tensor(m1[:], m1[:], m2[:], op=amx)
    ve.tensor_tensor(M1[:], M1[:], M2[:], op=amin)
    ve.tensor_tensor(t[:], m1[:], M1[:], op=amin)
    # t4: threshold broadcast over the group-of-4 axis
    t4 = pool.tile([P, nb, RANK], f16, tag="t4", name="t4")
    h4 = t4[:].rearrange("p b (g q) -> p b g q", q=4)
    for j in range(4):
        ve.tensor_copy(h4[:, :, :, j : j + 1], t[:])
    # s = w - clip(w, -t, t)
    th = thr[:]
    nt4 = pool.tile([P, nb, RANK], f16, tag="wneg", name="nt4")
    ve.tensor_scalar_mul(nt4[:], t4[:], -1.0)
    ve.tensor_tensor(th, wfh, t4[:], op=amin)
    ve.tensor_tensor(th, th, nt4[:], op=amx)
    ve.tensor_sub(th, wfh, th)
    if scale != 1.0:
        ve.tensor_scalar_mul(th, th, float(scale))
    if out_f16 is not None:
        ve.tensor_copy(out_f16[:], thr[:])
    return thr


def _build(scale_a, scale_b):
    import concourse.mybir as mybir
    import concourse.tile as tile
    from concourse import bacc
    from concourse.bass import ts
    from concourse.masks import make_identity

    f32, f16 = mybir.dt.float32, mybir.dt.float16

    nc = bacc.Bacc("TRN2", target_bir_lowering=False, debug=False,
                   enable_asserts=False)
    x_d = nc.dram_tensor("x", [T_CORE, IN_F], f16, kind="ExternalInput")
    wa_d = nc.dram_tensor("weight_A", [IN_F, RANK], f16, kind="ExternalInput")
    wb_d = nc.dram_tensor("weight_B", [OUT_F, RANK], f16, kind="ExternalInput")
    b_d = nc.dram_tensor("bias", [1, OUT_F], f16, kind="ExternalInput")
    o_d = nc.dram_tensor("out", [T_CORE, OUT_F], f16, kind="ExternalOutput")

    with tile.TileContext(nc) as tc:
        with (
            tc.tile_pool(name="const", bufs=1) as constp,
            tc.tile_pool(name="wtmp", bufs=1) as wtmp,
            tc.tile_pool(name="xtp", bufs=3) as xtp,
            tc.tile_pool(name="outp", bufs=3) as outp,
            tc.tile_pool(name="proj", bufs=2) as projp,
            tc.tile_pool(name="ps1", bufs=2, space="PSUM") as ps1p,
            tc.tile_pool(name="ps2", bufs=4, space="PSUM") as ps2p,
            tc.tile_pool(name="psw", bufs=1, space="PSUM") as pswp,
        ):
            ident16 = constp.tile([P, P], f16)
            make_identity(nc, ident16[:])

            # --- weight A first (mm1 of group 0 only needs wa16) ---
            wa16 = constp.tile([P, N_IB, RANK], f16)
            _soft_threshold_weights(nc, wtmp, wa_d, scale_a, wa16)

            # --- weight B: threshold, transpose on PE ---
            wbt = constp.tile([RANK + 1, OUT_F], f16)  # wB.T (+ bias row)
            thr_b = _soft_threshold_weights(nc, wtmp, wb_d, scale_b, None)
            for b in range(OUT_F // P):
                pw = pswp.tile([RANK, P], f16, tag="psw", name="pw")
                nc.tensor.transpose(pw[:], thr_b[:, b, :], ident16[:])
                nc.scalar.copy(wbt[0:RANK, ts(b, P)], pw[:])
            # bias row (row RANK)
            bstage = wtmp.tile([1, OUT_F], f16, tag="bstage", name="bstage")
            nc.scalar.dma_start(bstage[:], b_d[:])
            nc.vector.tensor_copy(wbt[RANK : RANK + 1, :], bstage[:])

            # --- main loop: 4 groups of 512 tokens ---
            for g in range(N_GRP):
                # XBAR-transposed loads: xT[:, c, t] = x[g*GTOK+t, c*128+p].
                # All XBAR transposes must stay on ONE queue: concurrent
                # transposes from both HWDGE queues corrupt each other (the
                # XBAR is a shared per-core resource).
                xT = xtp.tile([P, N_IB, GTOK], f16)
                for c in range(N_IB):
                    nc.sync.dma_start(xT[:, c, :],
                                      x_d[ts(g, GTOK), ts(c, P)],
                                      transpose=True)

                # mm1: x_projT[r, t] = sum_i wa[i, r] * xT[i, t]
                ps1 = ps1p.tile([RANK, GTOK], f32)
                for b in range(N_IB):
                    nc.tensor.matmul(ps1[:], wa16[:, b, :], xT[:, b, :],
                                     start=(b == 0), stop=(b == N_IB - 1))

                xpa = projp.tile([RANK + 1, GTOK], f16)
                nc.scalar.copy(xpa[0:RANK, :], ps1[:])
                nc.vector.memset(xpa[RANK : RANK + 1, :], 1.0)

                # mm2 per token tile: out[t, o] = x_projT.T @ wbt (+ bias row)
                for tt in range(TPG):
                    i = g * TPG + tt
                    ob = outp.tile([P, OUT_F], f16, name="ob", tag="ob")
                    for j in range(N_OB):
                        ps2 = ps2p.tile([P, MM2_N], f32, tag="ps2", name="ps2")
                        nc.tensor.matmul(ps2[:], xpa[:, ts(tt, P)],
                                         wbt[:, ts(j, MM2_N)],
                                         start=True, stop=True)
                        if j % 2 == 0:
                            nc.vector.tensor_copy(ob[:, ts(j, MM2_N)], ps2[:])
                        else:
                            nc.scalar.copy(ob[:, ts(j, MM2_N)], ps2[:])
                    nc.scalar.dma_start(o_d[ts(i, P), :], ob[:])

    nc.compile()
    return nc


def get_nc(scale_a, scale_b):
    key = (float(scale_a), float(scale_b))
    if key not in _CACHE:
        _CACHE[key] = _build(*key)
    return _CACHE[key]


def make_in_maps(x, weight_A, weight_B, bias):
    """Host-side shard + f16 cast: per-core input dicts."""
    x16 = np.ascontiguousarray(np.asarray(x, dtype=np.float32).astype(np.float16))
    wa = np.ascontiguousarray(np.asarray(weight_A, np.float32).astype(np.float16))
    wb = np.ascontiguousarray(np.asarray(weight_B, np.float32).astype(np.float16))
    bi = np.ascontiguousarray(
        np.asarray(bias, np.float32).astype(np.float16)).reshape(1, OUT_F)
    xf = x16.reshape(N_TOK, IN_F)
    return [
        {
            "x": xf[c * T_CORE : (c + 1) * T_CORE],
            "weight_A": wa,
            "weight_B": wb,
            "bias": bi,
        }
        for c in range(N_CORES)
    ]


def kernel(x, weight_A, weight_B, bias, scale_A, scale_B):
    from concourse.bass_utils import run_bass_kernel_spmd

    sa = float(np.asarray(scale_A))
    sb = float(np.asarray(scale_B))
    nc = get_nc(sa, sb)

    in_maps = make_in_maps(x, weight_A, weight_B, bias)
    res = run_bass_kernel_spmd(nc, in_maps, core_ids=list(range(N_CORES)))
    out = np.concatenate([r["out"] for r in res.results], axis=0)
    return out.astype(np.float32).reshape(B, S, OUT_F)


# revision 8
# speedup vs baseline: 1.0983x; 1.0983x over previous
"""Trainium2 Bass kernel: low-rank (LoRA-style) linear with 2:4 soft-threshold
pruned weights, fp16 matmul / fp32 accumulate.

  wA = soft_threshold24(weight_A) * scale_A          # [IN, R]
  wB = soft_threshold24(weight_B) * scale_B          # [OUT, R]
  x_proj = f16(x) @ f16(wA)            (f32 accum)   # [N, R]
  out    = f16(x_proj) @ f16(wB).T + bias            # [N, OUT]

Sharding: data-parallel over the token dim across 8 cores (2048 tokens/core),
small weights replicated. No collectives.

I/O: the reference computes both matmuls on f16 operands, so the host casts
x / weights / bias to f16 before upload and upcasts the f16 result to f32
after download. This halves HBM traffic (memory-bound kernel) and matches
reference numerics.

x-transpose strategy (mm1 needs x with IN on partitions):
  - chunks 0..K_XBAR-1: DMA XBAR transpose straight from DRAM, full token
    width [2048, 128] per instruction (~2us each, ~290 GB/s). All XBAR
    transposes stay on ONE queue (SP) - concurrent transposes from both
    HWDGE queues corrupt each other.
  - chunks K_XBAR..31: bulk f16 row loads + PE transposes (f16 identity,
    1 cyc/row), packed 8 per f16 PSUM bank, GPSIMD copies PSUM->SBUF.
mm1 for the XBAR chunks runs chunk-major into 4 persistent PSUM
accumulators so it trails the XBAR stream with no per-group barrier.
mm2 + f16 stores (on SP, after the xbars) trail per token tile; PSUM->SBUF
output copies round-robin across DVE/ACT/GPSIMD.
"""

import sys

import numpy as np

if "/opt/trn_rl_repo" not in sys.path:
    sys.path.insert(0, "/opt/trn_rl_repo")

B, S, IN_F, OUT_F, RANK = 4, 4096, 4096, 4096, 64
N_CORES = 8
N_TOK = B * S                   # 16384
T_CORE = N_TOK // N_CORES       # 2048 tokens per core
P = 128
GTOK = 512                      # tokens per group
N_GRP = T_CORE // GTOK          # 4 groups per core
TPG = GTOK // P                 # 4 token tiles per group
N_IB = IN_F // P                # 32 input-feature chunks
K_XBAR = 24                     # chunks via DMA XBAR transpose
N_PE = N_IB - K_XBAR            # chunks via PE transpose
MM2_N = 512
N_OB = OUT_F // MM2_N           # 8 output column groups

_CACHE = {}


def _soft_threshold_weights(nc, pool, w_dram, scale, out_f16):
    """Emit IR computing soft_threshold24(w_dram f16) * scale (f16).

    w_dram: [IN_or_OUT, RANK] f16, viewed as [P, blocks, RANK] with
    partition = row-within-block.
    """
    import concourse.mybir as mybir

    f16 = mybir.dt.float16
    nb = w_dram.shape[0] // P
    wf = pool.tile([P, nb, RANK], f16, tag="wstage", name="wstage")
    nc.scalar.dma_start(wf[:], w_dram[:].rearrange("(b p) r -> p b r", p=P))

    thr = pool.tile([P, nb, RANK], f16, tag="wthr", name="wthr")
    amin = mybir.AluOpType.min
    amx = mybir.AluOpType.max
    ve = nc.vector

    wfh = wf[:]
    g = wfh.rearrange("p b (g q) -> p b g q", q=4)
    gj = [g[:, :, :, j : j + 1] for j in range(4)]
    ash = [P, nb, RANK // 4, 1]
    wneg = pool.tile([P, nb, RANK], f16, tag="wneg", name="wneg")
    ve.tensor_scalar_mul(wneg[:], wfh, -1.0)
    ng = wneg[:].rearrange("p b (g q) -> p b g q", q=4)
    ab = [pool.tile(ash, f16, tag=f"abs{j}", name=f"abs{j}") for j in range(4)]
    for j in range(4):
        ve.tensor_tensor(ab[j][:], gj[j], ng[:, :, :, j : j + 1], op=amx)
    m1 = pool.tile(ash, f16, tag="m1", name="m1")
    M1 = pool.tile(ash, f16, tag="M1", name="M1")
    m2 = pool.tile(ash, f16, tag="abs0", name="m2")
    M2 = pool.tile(ash, f16, tag="abs1", name="M2")
    ve.tensor_tensor(m1[:], ab[0][:], ab[1][:], op=amin)
    ve.tensor_tensor(M1[:], ab[0][:], ab[1][:], op=amx)
    ve.tensor_tensor(m2[:], ab[2][:], ab[3][:], op=amin)
    ve.tensor_tensor(M2[:], ab[2][:], ab[3][:], op=amx)
    # 2nd smallest of the 4 = min(max(m1, m2), min(M1, M2))
    t = pool.tile(ash, f16, tag="abs2", name="t")
    ve.tensor_tensor(m1[:], m1[:], m2[:], op=amx)
    ve.tensor_tensor(M1[:], M1[:], M2[:], op=amin)
    ve.tensor_tensor(t[:], m1[:], M1[:], op=amin)
    # t4: threshold broadcast over the group-of-4 axis
    t4 = pool.tile([P, nb, RANK], f16, tag="t4", name="t4")
    h4 = t4[:].rearrange("p b (g q) -> p b g q", q=4)
    for j in range(4):
        ve.tensor_copy(h4[:, :, :, j : j + 1], t[:])
    # s = w - clip(w, -t, t)
    th = thr[:]
    nt4 = pool.tile([P, nb, RANK], f16, tag="wneg", name="nt4")
    ve.tensor_scalar_mul(nt4[:], t4[:], -1.0)
    ve.tensor_tensor(th, wfh, t4[:], op=amin)
    ve.tensor_tensor(th, th, nt4[:], op=amx)
    ve.tensor_sub(th, wfh, th)
    if scale != 1.0:
        ve.tensor_scalar_mul(th, th, float(scale))
    if out_f16 is not None:
        ve.tensor_copy(out_f16[:], thr[:])
    return thr


def _build(scale_a, scale_b):
    import concourse.mybir as mybir
    import concourse.tile as tile
    from concourse import bacc
    from concourse.bass import ts
    from concourse.masks import make_identity

    f32, f16 = mybir.dt.float32, mybir.dt.float16

    nc = bacc.Bacc("TRN2", target_bir_lowering=False, debug=False,
                   enable_asserts=False)
    x_d = nc.dram_tensor("x", [T_CORE, IN_F], f16, kind="ExternalInput")
    wa_d = nc.dram_tensor("weight_A", [IN_F, RANK], f16, kind="ExternalInput")
    wb_d = nc.dram_tensor("weight_B", [OUT_F, RANK], f16, kind="ExternalInput")
    b_d = nc.dram_tensor("bias", [1, OUT_F], f16, kind="ExternalInput")
    o_d = nc.dram_tensor("out", [T_CORE, OUT_F], f16, kind="ExternalOutput")

    with tile.TileContext(nc) as tc:
        with (
            tc.tile_pool(name="const", bufs=1) as constp,
            tc.tile_pool(name="wtmp", bufs=1) as wtmp,
            tc.tile_pool(name="xbig", bufs=1) as xbigp,
            tc.tile_pool(name="bulk", bufs=1) as bulkp,
            tc.tile_pool(name="xtpe", bufs=2) as xtpep,
            tc.tile_pool(name="outp", bufs=2) as outp,
            tc.tile_pool(name="proj", bufs=2) as projp,
            tc.tile_pool(name="ps1", bufs=4, space="PSUM") as ps1p,
            tc.tile_pool(name="pst", bufs=2, space="PSUM") as pstp,
            tc.tile_pool(name="ps2", bufs=2, space="PSUM") as ps2p,
        ):
            # --- DMA front matter: weights on ACT queue, xbars on SP ---
            # (emission order on each engine queue = execution order)
            ident16 = constp.tile([P, P], f16)
            make_identity(nc, ident16[:])

            # 24 full-token-width XBAR transposes: xbig[:, c, t] = x[t, c*128+p]
            xbig = xbigp.tile([P, K_XBAR, T_CORE], f16)
            for c in range(K_XBAR):
                nc.sync.dma_start(xbig[:, c, :], x_d[:, ts(c, P)],
                                  transpose=True)

            # --- weight A (mm1 needs wa16 early; wa DMA leads the ACT queue)
            wa16 = constp.tile([P, N_IB, RANK], f16)
            _soft_threshold_weights(nc, wtmp, wa_d, scale_a, wa16)

            # --- weight B threshold (transposed to wbt later, on PE) ---
            wbt = constp.tile([RANK + 1, OUT_F], f16)  # wB.T (+ bias row)
            thr_b = _soft_threshold_weights(nc, wtmp, wb_d, scale_b, None)
            bstage = wtmp.tile([1, OUT_F], f16, tag="bstage", name="bstage")
            nc.scalar.dma_start(bstage[:], b_d[:])
            nc.vector.tensor_copy(wbt[RANK : RANK + 1, :], bstage[:])

            # --- bulk f16 loads of the PE-transposed column block ---
            bulk = bulkp.tile([P, T_CORE // P, N_PE * P], f16)
            for i in range(T_CORE // P):
                nc.scalar.dma_start(bulk[:, i, :],
                                    x_d[ts(i, P), K_XBAR * P : IN_F])

            # --- per group: PE transposes + partial mm1 (PE chunks) ---
            accs = [ps1p.tile([RANK, GTOK], f32, tag="acc", name=f"acc{g}")
                    for g in range(N_GRP)]
            xtpes = []
            for g in range(N_GRP):
                xtpe = xtpep.tile([P, N_PE, GTOK], f16)
                xtpes.append(xtpe)
                for k in range(N_PE // 2):   # 2 chunks + 4 tt per PSUM bank
                    pt = pstp.tile([P, 2, TPG, P], f16, tag="ptx", name="pt")
                    for c2 in range(2):
                        cc = 2 * k + c2
                        for tt in range(TPG):
                            i = g * TPG + tt
                            nc.tensor.transpose(
                                pt[:, c2, tt, :],
                                bulk[:, i, ts(cc, P)], ident16[:])
                    # f16->f16 packed: DVE runs this at 2x (GPSIMD cannot
                    # read PSUM on TRN2 hardware)
                    nc.vector.tensor_copy(
                        xtpe[:, 2 * k : 2 * k + 2, :],
                        pt[:].rearrange("p a b c -> p a (b c)"))
                for cc in range(N_PE):
                    nc.tensor.matmul(accs[g][:], wa16[:, K_XBAR + cc, :],
                                     xtpe[:, cc, :],
                                     start=(cc == 0), stop=False)

            # --- wbt transposes on PE (thr_b ready well before this) ---
            for b in range(OUT_F // P):
                pw = pstp.tile([P, 2, TPG, P], f16, tag="ptx", name="pw")
                nc.tensor.transpose(pw[:RANK, 0, 0, :], thr_b[:, b, :],
                                    ident16[:])
                nc.vector.tensor_copy(wbt[0:RANK, ts(b, P)], pw[:RANK, 0, 0, :])

            # --- chunk-major mm1 over the XBAR chunks: trails the XBAR
            # stream; all 4 group accumulators live in PSUM ---
            for c in range(K_XBAR):
                for g in range(N_GRP):
                    nc.tensor.matmul(accs[g][:], wa16[:, c, :],
                                     xbig[:, c, ts(g, GTOK)],
                                     start=False, stop=(c == K_XBAR - 1))

            # --- mm2 + stores per group ---
            for g in range(N_GRP):
                xpa = projp.tile([RANK + 1, GTOK], f16)
                nc.scalar.copy(xpa[0:RANK, :], accs[g][:])
                nc.vector.memset(xpa[RANK : RANK + 1, :], 1.0)

                for tt in range(TPG):
                    i = g * TPG + tt
                    ob = outp.tile([P, OUT_F], f16, name="ob", tag="ob")
                    for j in range(N_OB):
                        ps2 = ps2p.tile([P, MM2_N], f32, tag="ps2", name="ps2")
                        nc.tensor.matmul(ps2[:], xpa[:, ts(tt, P)],
                                         wbt[:, ts(j, MM2_N)],
                                         start=True, stop=True)
                        dst = ob[:, ts(j, MM2_N)]
                        if j in (0, 3, 6):
                            nc.vector.tensor_copy(dst, ps2[:])
                        else:
                            nc.scalar.copy(dst, ps2[:])
                    nc.sync.dma_start(o_d[ts(i, P), :], ob[:])

    nc.compile()
    return nc


def get_nc(scale_a, scale_b):
    key = (float(scale_a), float(scale_b))
    if key not in _CACHE:
        _CACHE[key] = _build(*key)
    return _CACHE[key]


def make_in_maps(x, weight_A, weight_B, bias):
    """Host-side shard + f16 cast: per-core input dicts."""
    x16 = np.ascontiguousarray(np.asarray(x, dtype=np.float32).astype(np.float16))
    wa = np.ascontiguousarray(np.asarray(weight_A, np.float32).astype(np.float16))
    wb = np.ascontiguousarray(np.asarray(weight_B, np.float32).astype(np.float16))
    bi = np.ascontiguousarray(
        np.asarray(bias, np.float32).astype(np.float16)).reshape(1, OUT_F)
    xf = x16.reshape(N_TOK, IN_F)
    return [
        {
            "x": xf[c * T_CORE : (c + 1) * T_CORE],
            "weight_A": wa,
            "weight_B": wb,
            "bias": bi,
        }
        for c in range(N_CORES)
    ]


def kernel(x, weight_A, weight_B, bias, scale_A, scale_B):
    from concourse.bass_utils import run_bass_kernel_spmd

    sa = float(np.asarray(scale_A))
    sb = float(np.asarray(scale_B))
    nc = get_nc(sa, sb)

    in_maps = make_in_maps(x, weight_A, weight_B, bias)
    res = run_bass_kernel_spmd(nc, in_maps, core_ids=list(range(N_CORES)))
    out = np.concatenate([r["out"] for r in res.results], axis=0)
    return out.astype(np.float32).reshape(B, S, OUT_F)
